# revision 17
# baseline (speedup 1.0000x reference)
"""Trainium2 Bass kernel for the 3-block invertible coupling flow (RealNVP-style).

Computation (per sample row of x = [u1(256) | u2(256) | t(1)]):
    for j in 3 blocks:
        v1 = u1 * exp(mlp_s2(u2)) + mlp_t2(u2)
        v2 = u2 * exp(mlp_s1(v1)) + mlp_t1(v1)
        u1, u2 = v1, v2
    out = [u1 | u2 | t]
Each mlp is 256 -> 32 (tanh) -> 256.

Strategy: pure data parallel over batch (131072 -> 8 cores x 16384).
On-chip the state is kept feature-major ([128 feat partitions, batch free dim])
so every matmul streams the batch as the moving operand.  Input/output rows
are transposed with the tensor engine (exact fp32 identity transpose).
tanh/exp run on the scalar engine with the layer biases folded in as
per-partition activation biases; the final "*exp + t" uses one GPSIMD multiply
plus one DVE scalar_tensor_tensor which folds the t-MLP output bias and the
PSUM read into a single pass.

TRN2 instructions carry at most ONE sync wait (walrus fails with "Too many
sync wait commands" otherwise), and Tile's wait elision only credits an
engine's vector clock through *native* tile-access dependencies.  So before
each instruction that would need two cross-engine waits we emit a tiny
"leader" op on the same engine that natively reads the extra dependency
(matmul corners on the PE; [1,1] copies on ACT/DVE/GPSIMD).  Leaders write
into corners of the tile their group is about to overwrite, which both
orders them ahead of the group (WAW) and absorbs psum-slot release deps.
"""

from collections import deque
from contextlib import ExitStack

import numpy as np

import concourse.bass as bass
import concourse.tile as tile
from concourse import bacc, mybir
from concourse.bass_utils import run_bass_kernel_spmd

F32 = mybir.dt.float32
F32R = mybir.dt.float32r

# float32r (TF32-like) runs the PE at full rate but rounds inputs to ~10-bit
# mantissa; this flow chaotically amplifies matmul noise, so default to exact
# fp32 (4 cycles/row) until the fp32r error is measured on hardware.
USE_F32R = False

# With Bacc.compile() splitting multi-waits into event-semaphore chains, the
# leader ops below may be redundant; keep them switchable for A/B timing.
USE_LEADS = True


def _mm_dt(ap):
    return ap.bitcast(F32R) if USE_F32R else ap


B_TOTAL = 131072
D = 512
S = 256
H = 32
L = 3
NCORES = 8
BT = 512  # batch columns per supertile


def _pack_weights(W1, b1, W2, b2):
    """Host-side repack of the tiny MLP weights into PE-friendly layouts.

    q=0 updates u1 from u2 (s-idx 1, t-idx 3); q=1 updates u2 from v1
    (s-idx 0, t-idx 2).  Hidden units are duplicated [s,s,t,t] so the four
    K=32 second-layer matmuls can occupy the four 32-row PE groups.
    """
    W1 = np.asarray(W1, np.float32)
    b1 = np.asarray(b1, np.float32)
    W2 = np.asarray(W2, np.float32)
    b2 = np.asarray(b2, np.float32)
    w1p = np.empty((L, 2, 2, 128, 128), np.float32)
    b1p = np.empty((L, 2, 128), np.float32)
    w2p = np.empty((L, 2, 128, 256), np.float32)
    b2sp = np.empty((L, 2, 128, 2), np.float32)
    b2tp = np.empty((L, 2, 128, 2), np.float32)
    for j in range(L):
        for q in range(2):
            s_idx, t_idx = (1, 3) if q == 0 else (0, 2)
            for c in range(2):
                blk = slice(c * 128, (c + 1) * 128)
                w1p[j, q, c, :, 0:32] = W1[j, s_idx, blk, :]
                w1p[j, q, c, :, 32:64] = W1[j, s_idx, blk, :]
                w1p[j, q, c, :, 64:96] = W1[j, t_idx, blk, :]
                w1p[j, q, c, :, 96:128] = W1[j, t_idx, blk, :]
            b1p[j, q, 0:32] = b1[j, s_idx]
            b1p[j, q, 32:64] = b1[j, s_idx]
            b1p[j, q, 64:96] = b1[j, t_idx]
            b1p[j, q, 96:128] = b1[j, t_idx]
            w2p[j, q, 0:32, :] = W2[j, s_idx]
            w2p[j, q, 32:64, :] = W2[j, s_idx]
            w2p[j, q, 64:96, :] = W2[j, t_idx]
            w2p[j, q, 96:128, :] = W2[j, t_idx]
            b2sp[j, q, :, 0] = b2[j, s_idx, 0:128]
            b2sp[j, q, :, 1] = b2[j, s_idx, 128:256]
            b2tp[j, q, :, 0] = b2[j, t_idx, 0:128]
            b2tp[j, q, :, 1] = b2[j, t_idx, 128:256]
    return dict(w1p=w1p, b1p=b1p, w2p=w2p, b2sp=b2sp, b2tp=b2tp)


def build_nc(bc):
    """Build the per-core Bass program for a batch shard of `bc` rows."""
    assert bc % BT == 0
    nt = bc // BT
    nc = bacc.Bacc(None, target_bir_lowering=False)
    x_d = nc.declare_dram_parameter("x", [bc, D + 1], F32, isOutput=False)
    w1_d = nc.declare_dram_parameter("w1p", [L, 2, 2, 128, 128], F32, isOutput=False)
    b1_d = nc.declare_dram_parameter("b1p", [L, 2, 128], F32, isOutput=False)
    w2_d = nc.declare_dram_parameter("w2p", [L, 2, 128, 256], F32, isOutput=False)
    b2s_d = nc.declare_dram_parameter("b2sp", [L, 2, 128, 2], F32, isOutput=False)
    b2t_d = nc.declare_dram_parameter("b2tp", [L, 2, 128, 2], F32, isOutput=False)
    out_d = nc.declare_dram_parameter("out", [bc, D + 1], F32, isOutput=True)

    TANH = mybir.ActivationFunctionType.Tanh
    EXP = mybir.ActivationFunctionType.Exp
    ADD = mybir.AluOpType.add

    with tile.TileContext(nc) as tc, ExitStack() as ctx:
        singles = ctx.enter_context(tc.tile_pool(name="singles", bufs=1))
        p_xb = ctx.enter_context(tc.tile_pool(name="xb", bufs=2))
        p_state = ctx.enter_context(tc.tile_pool(name="state", bufs=4))
        p_th = ctx.enter_context(tc.tile_pool(name="th", bufs=2))
        p_e = ctx.enter_context(tc.tile_pool(name="e", bufs=2))
        p_tmp = ctx.enter_context(tc.tile_pool(name="tmp", bufs=2))
        p_outb = ctx.enter_context(tc.tile_pool(name="outb", bufs=2))
        ps_tr = ctx.enter_context(
            tc.tile_pool(name="ps_tr", bufs=2, space=bass.MemorySpace.PSUM)
        )
        ps_h = ctx.enter_context(
            tc.tile_pool(name="ps_h", bufs=2, space=bass.MemorySpace.PSUM)
        )
        ps_s = ctx.enter_context(
            tc.tile_pool(name="ps_s", bufs=1, space=bass.MemorySpace.PSUM)
        )
        ps_t = ctx.enter_context(
            tc.tile_pool(name="ps_t", bufs=1, space=bass.MemorySpace.PSUM)
        )

        def lead(ps_tile_ap, read_a, read_b=None):
            """Tiny PE matmul into a psum-tile corner; natively observes its
            reads on the PE vector clock."""
            if not USE_LEADS:
                return None
            rhs = read_b if read_b is not None else read_a
            return nc.tensor.matmul(
                ps_tile_ap[0:1, 0:1], read_a, rhs, start=True, stop=True
            )

        # --- constants / weights ----------------------------------------
        ident = singles.tile([128, 128], F32)
        nc.gpsimd.memset(ident[:], 0.0)
        nc.gpsimd.affine_select(
            out=ident[:],
            in_=ident[:],
            compare_op=mybir.AluOpType.not_equal,
            fill=1.0,
            base=0,
            pattern=[[-1, 128]],
            channel_multiplier=1,
        )
        w1s = singles.tile([128, L, 2, 2, 128], F32)
        nc.gpsimd.dma_start(
            out=w1s[:], in_=w1_d[:].rearrange("j q c p m -> p j q c m")
        )
        b1s = singles.tile([128, L, 2], F32)
        nc.gpsimd.dma_start(out=b1s[:], in_=b1_d[:].rearrange("j q p -> p j q"))
        w2s = singles.tile([128, L, 2, 256], F32)
        nc.gpsimd.dma_start(
            out=w2s[:], in_=w2_d[:].rearrange("j q p m -> p j q m")
        )
        b2ss = singles.tile([128, L, 2, 2], F32)
        nc.gpsimd.dma_start(out=b2ss[:], in_=b2s_d[:].rearrange("j q p c -> p j q c"))
        b2ts = singles.tile([128, L, 2, 2], F32)
        nc.gpsimd.dma_start(out=b2ts[:], in_=b2t_d[:].rearrange("j q p c -> p j q c"))

        # startup PE leaders: observe ident + weight DMAs on the PE clock
        scr = ps_h.tile([128, BT], F32, tag="h")
        lead(scr[:], ident[:, 0:1])
        lead(scr[:], w1s[:, 0, 0, 0, 0:1])
        lead(scr[:], w2s[:, 0, 0, 0:1])

        # non-PE leaders write rotating, never-overlapping columns of
        # per-engine scratch tiles: a [1,1] copy natively observes its input
        # on that engine's vector clock while creating no WAW hazards (which
        # would cost a same-engine sync wait of their own).
        n_cols = 16 * (nt + 2)
        act_scr = singles.tile([1, n_cols], F32)
        dve_scr = singles.tile([1, n_cols], F32)
        pool_scr = singles.tile([1, n_cols], F32)
        cols = {"act": 0, "dve": 0, "pool": 0}

        def act_lead(src_ap):
            if not USE_LEADS:
                return
            c = cols["act"]; cols["act"] += 1
            nc.scalar.copy(out=act_scr[0:1, c : c + 1], in_=src_ap)

        def dve_lead(src_ap):
            if not USE_LEADS:
                return
            c = cols["dve"]; cols["dve"] += 1
            nc.vector.tensor_copy(out=dve_scr[0:1, c : c + 1], in_=src_ap)

        def pool_lead(a_ap, b_ap):
            if not USE_LEADS:
                return None
            c = cols["pool"]; cols["pool"] += 1
            nc.gpsimd.tensor_add(
                out=pool_scr[0:1, c : c + 1], in0=a_ap, in1=b_ap
            )
            return c

        # startup: observe the bias DMAs on the engines that read them
        act_lead(b1s[0:1, 0, 0:1])
        act_lead(b2ss[0:1, 0, 0, 0:1])
        act_lead(b2ts[0:1, 0, 0, 0:1])
        dve_lead(b2ts[0:1, 0, 0, 0:1])

        # old-tile handles per psum tag: group_leads writes a corner of the
        # OLD tile (native WAR dep on its cross-engine reader), then
        # first-writes the NEW tile (carries only the PE bank-reuse wait),
        # then data leads, then the real matmuls.
        tr_old = deque(maxlen=2)
        h_old = deque([scr[:]], maxlen=2)
        s_old = deque(maxlen=1)
        t_old = deque(maxlen=1)

        def group_leads(old_deque, new_tile_ap, data_reads):
            if len(old_deque) == old_deque.maxlen:
                lead(old_deque[0], ident[:, 0:1])
            lead(new_tile_ap, ident[:, 0:1])
            for ra, rb in data_reads:
                lead(new_tile_ap, ra, rb)
            old_deque.append(new_tile_ap)

        tmp_prev = None
        u_prev = None

        for st in range(nt):
            b0 = st * BT
            xb = p_xb.tile([128, 4, D + 1], F32, tag="xb")
            nc.sync.dma_start(
                out=xb[:],
                in_=x_d[b0 : b0 + BT, :].rearrange("(r p) f -> p r f", p=128),
            )

            # transpose to feature-major state tiles u[h] = [128, fc, BT]
            u = []
            for h in range(2):
                ut = p_state.tile([128, 2, BT], F32, tag=f"state{h}{st % 2}")
                for fc in range(2):
                    f0 = h * 256 + fc * 128
                    ps = ps_tr.tile([128, BT], F32, tag="tr")
                    group_leads(tr_old, ps[:], [(xb[:, 0, 0:1], None)])
                    for rt in range(4):
                        nc.tensor.transpose(
                            ps[:, rt * 128 : (rt + 1) * 128],
                            xb[:, rt, f0 : f0 + 128],
                            ident[:],
                        )
                    nc.vector.tensor_copy(out=ut[:, fc, :], in_=ps[:])
                u.append(ut)
            if u_prev is not None:
                # Pool leaders: observe the previous supertile's final DVE
                # writes so the first muls' tmp-slot releases are covered.
                pool_lead(u_prev[0][0:1, 0, 0:1], u_prev[0][0:1, 1, 0:1])
                pool_lead(u_prev[1][0:1, 0, 0:1], u_prev[1][0:1, 1, 0:1])

            for j in range(L):
                for q in range(2):
                    hin = u[1 - q]
                    tgt = u[q]
                    # layer 1: hidden pair, duplicated [s,s,t,t] on partitions
                    ph = ps_h.tile([128, BT], F32, tag="h")
                    group_leads(
                        h_old, ph[:], [(hin[:, 0, 0:1], hin[:, 1, 0:1])]
                    )
                    for c in range(2):
                        nc.tensor.matmul(
                            ph[:],
                            _mm_dt(w1s[:, j, q, c, :]),
                            _mm_dt(hin[:, c, :]),
                            start=(c == 0),
                            stop=(c == 1),
                        )
                    th = p_th.tile([128, BT], F32, tag="th")
                    nc.scalar.activation(th[:], ph[:], TANH, bias=b1s[:, j, q : q + 1])
                    # layer 2: four K=32 matmuls on the four PE row groups
                    pss = ps_s.tile([128, 2, BT], F32, tag="s")
                    pst = ps_t.tile([128, 2, BT], F32, tag="t")
                    group_leads(s_old, pss[:, 0, :], [(th[:, 0:1], None)])
                    group_leads(t_old, pst[:, 0, :], [(th[:, 0:1], None)])
                    for fc in range(2):
                        r = 32 * fc
                        nc.tensor.matmul(
                            pss[:, fc, :],
                            _mm_dt(w2s[r : r + 32, j, q, fc * 128 : (fc + 1) * 128]),
                            _mm_dt(th[r : r + 32, :]),
                            tile_position=(r, 0),
                        )
                        r = 64 + 32 * fc
                        nc.tensor.matmul(
                            pst[:, fc, :],
                            _mm_dt(w2s[r : r + 32, j, q, fc * 128 : (fc + 1) * 128]),
                            _mm_dt(th[r : r + 32, :]),
                            tile_position=(r, 0),
                        )
                    # E = exp(s + b2s); tmp = tgt * E; v = (tmp + b2t) + t_psum
                    ee = p_e.tile([128, 2, BT], F32, tag="e")
                    for fc in range(2):
                        nc.scalar.activation(
                            ee[:, fc, :],
                            pss[:, fc, :],
                            EXP,
                            bias=b2ss[:, j, q, fc : fc + 1],
                        )
                    # tmp gets a padding column whose only writer is a tiny
                    # memset: as tmp's first created writer it alone carries
                    # the Pool-self slot-release wait, and the pool_lead
                    # observes tgt's DVE producers, so the multiply itself
                    # only waits on exp.
                    tmp = p_tmp.tile([128, 2, BT + 1], F32, tag="tmp")
                    pool_lead(tgt[0:1, 0, 0:1], tgt[0:1, 1, 0:1])
                    if USE_LEADS:
                        nc.gpsimd.memset(tmp[0:1, 0, BT : BT + 1], 0.0)
                        # ACT leader: the memset's padding column postdates
                        # the mul whose ee slot this half-step's exp reuses.
                        act_lead(tmp[0:1, 0, BT : BT + 1])
                    nc.gpsimd.tensor_mul(
                        out=tmp[:, :, 0:BT], in0=tgt[:], in1=ee[:]
                    )
                    v = p_state.tile([128, 2, BT], F32, tag=f"state{q}{st % 2}")
                    # DVE leader: observe the multiply so each
                    # scalar_tensor_tensor only waits on its own psum.
                    dve_lead(tmp[0:1, 0, 0:1])
                    for fc in range(2):
                        nc.vector.scalar_tensor_tensor(
                            out=v[:, fc, :],
                            in0=tmp[:, fc, 0:BT],
                            scalar=b2ts[:, j, q, fc : fc + 1],
                            in1=pst[:, fc, :],
                            op0=ADD,
                            op1=ADD,
                        )
                    u[q] = v

            u_prev = u
            # transpose back to batch-major and store.  outb gets a padding
            # column D+1 whose only writer is the release-absorbing ACT
            # leader (outb's first created writer), so the real copies carry
            # only their own PE wait and the t-col copy only the xb DMA wait.
            outb = p_outb.tile([128, 4, D + 2], F32, tag="outb")
            if USE_LEADS:
                nc.scalar.copy(out=outb[0:1, 0, D + 1 : D + 2], in_=b1s[0:1, 0, 0:1])
            for rt in range(4):
                po = ps_tr.tile([128, BT], F32, tag="tr")
                group_leads(
                    tr_old,
                    po[:],
                    [
                        (u[0][:, 0, 0:1], u[0][:, 1, 0:1]),
                        (u[1][:, 0, 0:1], u[1][:, 1, 0:1]),
                    ],
                )
                for fc4 in range(4):
                    h, fc = divmod(fc4, 2)
                    nc.tensor.transpose(
                        po[:, fc4 * 128 : (fc4 + 1) * 128],
                        u[h][:, fc, rt * 128 : (rt + 1) * 128],
                        ident[:],
                    )
                nc.scalar.copy(out=outb[:, rt, 0:D], in_=po[:])
                # PE cover-lead: observe this copy on the PE clock (written
                # into the retired s-psum corner) so the next groups reusing
                # the transpose slots only carry their PE bank wait.
                lead(pss[:, 0, :], outb[0:1, rt, 0:1])
            nc.scalar.copy(out=outb[:, :, D], in_=xb[:, :, D])
            nc.sync.dma_start(
                out=out_d[b0 : b0 + BT, :].rearrange("(r p) f -> p r f", p=128),
                in_=outb[:, :, 0 : D + 1],
            )
    nc.compile()
    return nc


def scan_waits(nc, limit=1):
    """Return instructions carrying more than `limit` sync waits."""
    bad = []
    for blk in nc.m.functions[0].blocks:
        for inst in blk.instructions:
            cls = inst.__class__.__name__
            if cls in ("InstDrain", "InstEventSemaphore"):
                continue
            si = inst.sync_info
            nw = len(si.on_wait) if si and si.on_wait else 0
            if nw > limit:
                bad.append(
                    (cls, inst.name, [(w.ant_name, w.wait_value) for w in si.on_wait])
                )
    return bad


_NC_CACHE = {}
TRACE = False
LAST_EXEC_NS = None


def _get_nc(bc):
    if bc not in _NC_CACHE:
        _NC_CACHE[bc] = build_nc(bc)
    return _NC_CACHE[bc]


def kernel(x, W1, b1, W2, b2):
    global LAST_EXEC_NS
    x = np.ascontiguousarray(np.asarray(x, np.float32))
    b = x.shape[0]
    assert b % NCORES == 0
    bc = b // NCORES
    packed = _pack_weights(W1, b1, W2, b2)
    nc = _get_nc(bc)
    in_maps = [
        {"x": x[i * bc : (i + 1) * bc], **packed} for i in range(NCORES)
    ]
    res = run_bass_kernel_spmd(nc, in_maps, list(range(NCORES)), trace=TRACE)
    if getattr(res, "exec_time_ns", None):
        LAST_EXEC_NS = res.exec_time_ns
    out = np.concatenate([res.results[i]["out"] for i in range(NCORES)], axis=0)
    return out.astype(np.float32)


# revision 23
# speedup vs baseline: 2.3002x; 2.3002x over previous
"""Trainium2 Bass kernel for the 3-block invertible coupling flow (RealNVP-style).

Computation (per sample row of x = [u1(256) | u2(256) | t(1)]):
    for j in 3 blocks:
        v1 = u1 * exp(mlp_s2(u2)) + mlp_t2(u2)
        v2 = u2 * exp(mlp_s1(v1)) + mlp_t1(v1)
        u1, u2 = v1, v2
    out = [u1 | u2 | t]
Each mlp is 256 -> 32 (tanh) -> 256.

Strategy: pure data parallel over batch (131072 -> 8 cores x 16384).
On-chip the state is kept feature-major ([128 feat partitions, batch free dim])
so every matmul streams the batch as the moving operand.  Input/output rows
are transposed with the tensor engine (exact fp32 identity transpose).
tanh/exp run on the scalar engine with the layer biases folded in as
per-partition activation biases; the final "*exp + t" uses one GPSIMD multiply
plus one DVE scalar_tensor_tensor which folds the t-MLP output bias and the
PSUM read into a single pass.

TRN2 instructions carry at most ONE sync wait (walrus fails with "Too many
sync wait commands" otherwise), and Tile's wait elision only credits an
engine's vector clock through *native* tile-access dependencies.  So before
each instruction that would need two cross-engine waits we emit a tiny
"leader" op on the same engine that natively reads the extra dependency
(matmul corners on the PE; [1,1] copies on ACT/DVE/GPSIMD).  Leaders write
into corners of the tile their group is about to overwrite, which both
orders them ahead of the group (WAW) and absorbs psum-slot release deps.
"""

from collections import deque
from contextlib import ExitStack

import numpy as np

import concourse.bass as bass
import concourse.tile as tile
from concourse import bacc, mybir
from concourse.bass_utils import run_bass_kernel_spmd

F32 = mybir.dt.float32
F32R = mybir.dt.float32r

# float32r (TF32-like) runs the PE at full rate but rounds inputs to ~10-bit
# mantissa; this flow chaotically amplifies matmul noise, so default to exact
# fp32 (4 cycles/row) until the fp32r error is measured on hardware.
USE_F32R = False

# With Bacc.compile() splitting multi-waits into event-semaphore chains, the
# leader ops below may be redundant; keep them switchable for A/B timing.
USE_LEADS = True

# Run only the K=32 second-layer matmuls in float32r (TF32-ish).  Their input
# is tanh output in [-1,1]; the ~2^-11 rounding there injects ~6e-5 absolute
# into the pre-exp activations, far below the flow's chaotic fp32 envelope.
USE_F32R_L2 = True


def _mm_dt(ap):
    return ap.bitcast(F32R) if USE_F32R else ap


B_TOTAL = 131072
D = 512
S = 256
H = 32
L = 3
NCORES = 8
BT = 512  # batch columns per supertile


def _pack_weights(W1, b1, W2, b2):
    """Host-side repack of the tiny MLP weights into PE-friendly layouts.

    q=0 updates u1 from u2 (s-idx 1, t-idx 3); q=1 updates u2 from v1
    (s-idx 0, t-idx 2).  Hidden units are duplicated [s,s,t,t] so the four
    K=32 second-layer matmuls can occupy the four 32-row PE groups.
    """
    W1 = np.asarray(W1, np.float32)
    b1 = np.asarray(b1, np.float32)
    W2 = np.asarray(W2, np.float32)
    b2 = np.asarray(b2, np.float32)
    w1p = np.empty((L, 2, 2, 128, 128), np.float32)
    b1p = np.empty((L, 2, 128), np.float32)
    w2p = np.empty((L, 2, 128, 256), np.float32)
    b2sp = np.empty((L, 2, 128, 2), np.float32)
    b2tp = np.empty((L, 2, 128, 2), np.float32)
    for j in range(L):
        for q in range(2):
            s_idx, t_idx = (1, 3) if q == 0 else (0, 2)
            for c in range(2):
                blk = slice(c * 128, (c + 1) * 128)
                w1p[j, q, c, :, 0:32] = W1[j, s_idx, blk, :]
                w1p[j, q, c, :, 32:64] = W1[j, s_idx, blk, :]
                w1p[j, q, c, :, 64:96] = W1[j, t_idx, blk, :]
                w1p[j, q, c, :, 96:128] = W1[j, t_idx, blk, :]
            b1p[j, q, 0:32] = b1[j, s_idx]
            b1p[j, q, 32:64] = b1[j, s_idx]
            b1p[j, q, 64:96] = b1[j, t_idx]
            b1p[j, q, 96:128] = b1[j, t_idx]
            w2p[j, q, 0:32, :] = W2[j, s_idx]
            w2p[j, q, 32:64, :] = W2[j, s_idx]
            w2p[j, q, 64:96, :] = W2[j, t_idx]
            w2p[j, q, 96:128, :] = W2[j, t_idx]
            b2sp[j, q, :, 0] = b2[j, s_idx, 0:128]
            b2sp[j, q, :, 1] = b2[j, s_idx, 128:256]
            b2tp[j, q, :, 0] = b2[j, t_idx, 0:128]
            b2tp[j, q, :, 1] = b2[j, t_idx, 128:256]
    return dict(w1p=w1p, b1p=b1p, w2p=w2p, b2sp=b2sp, b2tp=b2tp)


def build_nc(bc):
    """Build the per-core Bass program for a batch shard of `bc` rows."""
    assert bc % BT == 0
    nt = bc // BT
    nc = bacc.Bacc(None, target_bir_lowering=False)
    x_d = nc.declare_dram_parameter("x", [bc, D + 1], F32, isOutput=False)
    w1_d = nc.declare_dram_parameter("w1p", [L, 2, 2, 128, 128], F32, isOutput=False)
    b1_d = nc.declare_dram_parameter("b1p", [L, 2, 128], F32, isOutput=False)
    w2dt = F32R if USE_F32R_L2 else F32
    w2_d = nc.declare_dram_parameter("w2p", [L, 2, 128, 256], w2dt, isOutput=False)
    b2s_d = nc.declare_dram_parameter("b2sp", [L, 2, 128, 2], F32, isOutput=False)
    b2t_d = nc.declare_dram_parameter("b2tp", [L, 2, 128, 2], F32, isOutput=False)
    out_d = nc.declare_dram_parameter("out", [bc, D + 1], F32, isOutput=True)

    TANH = mybir.ActivationFunctionType.Tanh
    EXP = mybir.ActivationFunctionType.Exp
    ADD = mybir.AluOpType.add

    with tile.TileContext(nc) as tc, ExitStack() as ctx:
        singles = ctx.enter_context(tc.tile_pool(name="singles", bufs=1))
        p_xb = ctx.enter_context(tc.tile_pool(name="xb", bufs=2))
        p_state = ctx.enter_context(tc.tile_pool(name="state", bufs=4))
        p_th = ctx.enter_context(tc.tile_pool(name="th", bufs=4))
        p_e = ctx.enter_context(tc.tile_pool(name="e", bufs=3))
        p_tmp = ctx.enter_context(tc.tile_pool(name="tmp", bufs=3))
        p_outb = ctx.enter_context(tc.tile_pool(name="outb", bufs=2))
        ps_tri = ctx.enter_context(
            tc.tile_pool(name="ps_tri", bufs=2, space=bass.MemorySpace.PSUM)
        )
        ps_h = ctx.enter_context(
            tc.tile_pool(name="ps_h", bufs=2, space=bass.MemorySpace.PSUM)
        )
        ps_s = ctx.enter_context(
            tc.tile_pool(name="ps_s", bufs=1, space=bass.MemorySpace.PSUM)
        )
        ps_t = ctx.enter_context(
            tc.tile_pool(name="ps_t", bufs=1, space=bass.MemorySpace.PSUM)
        )

        def lead(ps_tile_ap, read_a, read_b=None):
            """Tiny PE matmul into a psum-tile corner; natively observes its
            reads on the PE vector clock."""
            if not USE_LEADS:
                return None
            rhs = read_b if read_b is not None else read_a
            return nc.tensor.matmul(
                ps_tile_ap[0:1, 0:1], read_a, rhs, start=True, stop=True
            )

        # --- constants / weights ----------------------------------------
        ident = singles.tile([128, 128], F32)
        nc.gpsimd.memset(ident[:], 0.0)
        nc.gpsimd.affine_select(
            out=ident[:],
            in_=ident[:],
            compare_op=mybir.AluOpType.not_equal,
            fill=1.0,
            base=0,
            pattern=[[-1, 128]],
            channel_multiplier=1,
        )
        w1s = singles.tile([128, L, 2, 2, 128], F32)
        nc.gpsimd.dma_start(
            out=w1s[:], in_=w1_d[:].rearrange("j q c p m -> p j q c m")
        )
        b1s = singles.tile([128, L, 2], F32)
        nc.gpsimd.dma_start(out=b1s[:], in_=b1_d[:].rearrange("j q p -> p j q"))
        w2s = singles.tile([128, L, 2, 256], w2dt)
        nc.gpsimd.dma_start(
            out=w2s[:], in_=w2_d[:].rearrange("j q p m -> p j q m")
        )
        b2ss = singles.tile([128, L, 2, 2], F32)
        nc.gpsimd.dma_start(out=b2ss[:], in_=b2s_d[:].rearrange("j q p c -> p j q c"))
        b2ts = singles.tile([128, L, 2, 2], F32)
        nc.gpsimd.dma_start(out=b2ts[:], in_=b2t_d[:].rearrange("j q p c -> p j q c"))

        # startup PE leaders: observe ident + weight DMAs on the PE clock
        scr = ps_h.tile([128, BT], F32, tag="h")
        lead(scr[:], ident[:, 0:1])
        lead(scr[:], w1s[:, 0, 0, 0, 0:1])
        lead(scr[:], w2s[:, 0, 0, 0:1])

        # non-PE leaders write rotating, never-overlapping columns of
        # per-engine scratch tiles: a [1,1] copy natively observes its input
        # on that engine's vector clock while creating no WAW hazards (which
        # would cost a same-engine sync wait of their own).
        n_cols = 16 * (nt + 2)
        act_scr = singles.tile([1, n_cols], F32)
        dve_scr = singles.tile([1, n_cols], F32)
        pool_scr = singles.tile([1, n_cols], F32)
        cols = {"act": 0, "dve": 0, "pool": 0}

        def act_lead(src_ap):
            if not USE_LEADS:
                return
            c = cols["act"]; cols["act"] += 1
            nc.scalar.copy(out=act_scr[0:1, c : c + 1], in_=src_ap)

        def dve_lead(src_ap):
            if not USE_LEADS:
                return
            c = cols["dve"]; cols["dve"] += 1
            nc.vector.tensor_copy(out=dve_scr[0:1, c : c + 1], in_=src_ap)

        def pool_lead(a_ap, b_ap):
            if not USE_LEADS:
                return None
            c = cols["pool"]; cols["pool"] += 1
            nc.gpsimd.tensor_add(
                out=pool_scr[0:1, c : c + 1], in0=a_ap, in1=b_ap
            )
            return c

        # startup: observe the bias DMAs on the engines that read them
        act_lead(b1s[0:1, 0, 0:1])
        act_lead(b2ss[0:1, 0, 0, 0:1])
        act_lead(b2ts[0:1, 0, 0, 0:1])
        dve_lead(b2ts[0:1, 0, 0, 0:1])

        # old-tile handles per psum tag: group_leads writes a corner of the
        # OLD tile (native WAR dep on its cross-engine reader), then
        # first-writes the NEW tile (carries only the PE bank-reuse wait),
        # then data leads, then the real matmuls.
        tr_old = deque(maxlen=2)
        h_old = deque([scr[:]], maxlen=2)
        s_old = deque(maxlen=1)
        t_old = deque(maxlen=1)

        def group_leads(old_deque, new_tile_ap, data_reads):
            if len(old_deque) == old_deque.maxlen:
                lead(old_deque[0], ident[:, 0:1])
            lead(new_tile_ap, ident[:, 0:1])
            for ra, rb in data_reads:
                lead(new_tile_ap, ra, rb)
            old_deque.append(new_tile_ap)

        tmp_prev = None
        u_prev = {}

        assert nt % 2 == 0
        for pair in range(nt // 2):
            sts = (2 * pair, 2 * pair + 1)
            xb = {}
            u = {}
            # load + in-transpose both chains of the pair
            for st in sts:
                b0 = st * BT
                xb[st] = p_xb.tile([128, 4, D + 1], F32, tag="xb", name=f"xbt{st}")
                nc.sync.dma_start(
                    out=xb[st][:],
                    in_=x_d[b0 : b0 + BT, :].rearrange("(r p) f -> p r f", p=128),
                )
                u[st] = []
                for h in range(2):
                    ut = p_state.tile([128, 2, BT], F32, tag=f"state{h}{st % 2}")
                    for fc in range(2):
                        f0 = h * 256 + fc * 128
                        ps = ps_tri.tile([128, BT], F32, tag="tri")
                        group_leads(tr_old, ps[:], [(xb[st][:, 0, 0:1], None)])
                        for rt in range(4):
                            nc.tensor.transpose(
                                ps[:, rt * 128 : (rt + 1) * 128],
                                xb[st][:, rt, f0 : f0 + 128],
                                ident[:],
                            )
                        nc.vector.tensor_copy(out=ut[:, fc, :], in_=ps[:])
                    u[st].append(ut)
                if u_prev.get(st % 2) is not None:
                    up = u_prev[st % 2]
                    pool_lead(up[0][0:1, 0, 0:1], up[0][0:1, 1, 0:1])
                    pool_lead(up[1][0:1, 0, 0:1], up[1][0:1, 1, 0:1])

            # interleave the two chains' half-steps so one chain's matmuls
            # overlap the other chain's activations/elementwise work
            for j in range(L):
                for q in range(2):
                    for st in sts:
                        hin = u[st][1 - q]
                        tgt = u[st][q]
                        ph = ps_h.tile([128, BT], F32, tag="h")
                        group_leads(
                            h_old, ph[:], [(hin[:, 0, 0:1], hin[:, 1, 0:1])]
                        )
                        for c in range(2):
                            nc.tensor.matmul(
                                ph[:],
                                _mm_dt(w1s[:, j, q, c, :]),
                                _mm_dt(hin[:, c, :]),
                                start=(c == 0),
                                stop=(c == 1),
                            )
                        th = p_th.tile([128, BT], w2dt, tag="th")
                        nc.scalar.activation(
                            th[:], ph[:], TANH, bias=b1s[:, j, q : q + 1]
                        )
                        pss = ps_s.tile([128, 2, BT], F32, tag="s")
                        pst = ps_t.tile([128, 2, BT], F32, tag="t")
                        group_leads(s_old, pss[:, 0, :], [(th[:, 0:1], None)])
                        group_leads(t_old, pst[:, 0, :], [(th[:, 0:1], None)])
                        for fc in range(2):
                            r = 32 * fc
                            nc.tensor.matmul(
                                pss[:, fc, :],
                                w2s[r : r + 32, j, q, fc * 128 : (fc + 1) * 128],
                                th[r : r + 32, :],
                                tile_position=(r, 0),
                            )
                            r = 64 + 32 * fc
                            nc.tensor.matmul(
                                pst[:, fc, :],
                                w2s[r : r + 32, j, q, fc * 128 : (fc + 1) * 128],
                                th[r : r + 32, :],
                                tile_position=(r, 0),
                            )
                        ee = p_e.tile([128, 2, BT], F32, tag="e")
                        for fc in range(2):
                            nc.scalar.activation(
                                ee[:, fc, :],
                                pss[:, fc, :],
                                EXP,
                                bias=b2ss[:, j, q, fc : fc + 1],
                            )
                        tmp = p_tmp.tile([128, 2, BT + 1], F32, tag="tmp")
                        pool_lead(tgt[0:1, 0, 0:1], tgt[0:1, 1, 0:1])
                        if USE_LEADS:
                            nc.gpsimd.memset(tmp[0:1, 0, BT : BT + 1], 0.0)
                            act_lead(tmp[0:1, 0, BT : BT + 1])
                        nc.vector.tensor_mul(
                            out=tmp[:, :, 0:BT], in0=tgt[:], in1=ee[:]
                        )
                        v = p_state.tile([128, 2, BT], F32, tag=f"state{q}{st % 2}")
                        dve_lead(tmp[0:1, 0, 0:1])
                        for fc in range(2):
                            nc.vector.scalar_tensor_tensor(
                                out=v[:, fc, :],
                                in0=tmp[:, fc, 0:BT],
                                scalar=b2ts[:, j, q, fc : fc + 1],
                                in1=pst[:, fc, :],
                                op0=ADD,
                                op1=ADD,
                            )
                        u[st][q] = v

            # output phase for both chains
            for st in sts:
                b0 = st * BT
                u_prev[st % 2] = u[st]
                outb = p_outb.tile([128, 4, D + 2], F32, tag="outb")
                if USE_LEADS:
                    nc.scalar.copy(
                        out=outb[0:1, 0, D + 1 : D + 2], in_=b1s[0:1, 0, 0:1]
                    )
                for rt in range(4):
                    po = ps_h.tile([128, BT], F32, tag="h")
                    group_leads(
                        tr_old,
                        po[:],
                        [
                            (u[st][0][:, 0, 0:1], u[st][0][:, 1, 0:1]),
                            (u[st][1][:, 0, 0:1], u[st][1][:, 1, 0:1]),
                        ],
                    )
                    for fc4 in range(4):
                        h, fc = divmod(fc4, 2)
                        nc.tensor.transpose(
                            po[:, fc4 * 128 : (fc4 + 1) * 128],
                            u[st][h][:, fc, rt * 128 : (rt + 1) * 128],
                            ident[:],
                        )
                    nc.scalar.copy(out=outb[:, rt, 0:D], in_=po[:])
                    lead(pss[:, 0, :], outb[0:1, rt, 0:1])
                nc.scalar.copy(out=outb[:, :, D], in_=xb[st][:, :, D])
                nc.sync.dma_start(
                    out=out_d[b0 : b0 + BT, :].rearrange("(r p) f -> p r f", p=128),
                    in_=outb[:, :, 0 : D + 1],
                )
    nc.compile()
    return nc


def scan_waits(nc, limit=1):
    """Return instructions carrying more than `limit` sync waits."""
    bad = []
    for blk in nc.m.functions[0].blocks:
        for inst in blk.instructions:
            cls = inst.__class__.__name__
            if cls in ("InstDrain", "InstEventSemaphore"):
                continue
            si = inst.sync_info
            nw = len(si.on_wait) if si and si.on_wait else 0
            if nw > limit:
                bad.append(
                    (cls, inst.name, [(w.ant_name, w.wait_value) for w in si.on_wait])
                )
    return bad


_NC_CACHE = {}
TRACE = False
LAST_EXEC_NS = None
LAST_RES = None


def _get_nc(bc):
    if bc not in _NC_CACHE:
        _NC_CACHE[bc] = build_nc(bc)
    return _NC_CACHE[bc]


def kernel(x, W1, b1, W2, b2):
    global LAST_EXEC_NS
    x = np.ascontiguousarray(np.asarray(x, np.float32))
    b = x.shape[0]
    assert b % NCORES == 0
    bc = b // NCORES
    packed = _pack_weights(W1, b1, W2, b2)
    nc = _get_nc(bc)
    in_maps = [
        {"x": x[i * bc : (i + 1) * bc], **packed} for i in range(NCORES)
    ]
    res = run_bass_kernel_spmd(nc, in_maps, list(range(NCORES)), trace=TRACE)
    if getattr(res, "exec_time_ns", None):
        LAST_EXEC_NS = res.exec_time_ns
    if TRACE:
        globals()["LAST_RES"] = res
    out = np.concatenate([res.results[i]["out"] for i in range(NCORES)], axis=0)
    return out.astype(np.float32)
